# revision 38
# baseline (speedup 1.0000x reference)
"""Trainium2 Bass kernel for nn_Net_74552042324489.

Data-parallel over batch n=8 across 8 NeuronCores (1 sample/core).
~195us measured (225us baseline).  Per-core pipeline:
  DMA: bulk inputs only on the two hardware DGE queues, interleaved in
       PE-consumption order (SP: x2/x4; ACT: weights -> xsb -> deep3 ->
       qk weights).  Host lays slabs out for 1.5-6KB descriptors (12KB
       descriptors serialize on single DMA engines; gpsimd queue is a
       slow software path used only for tiny SBUF moves).  deep3 pad
       rows are memset on-chip; the 1.6MB resize-coefficient tensor is
       built on-chip from its rank-1 factors (one broadcast multiply).
  cam  in fp16 hi/lo 3-term (Wh*H + Wl*H + Wh*L): fp32-level accuracy
       (needed for the argmax-suppression ties; bf16 pairs measurably
       flip them) at fp16 matmul rate.  x stays fp16: fp8 x pushes the
       output to 1.7e-2 via the q,k cross terms.
  x_s  emitted right after the first y2 chunk as PE filler; identity
       ships via DMA.  The 112->56 resize runs per-x2-chunk so f_b is
       ready as soon as the last x2 chunk lands.  f8_4 reuses the y2
       PSUM pool inside phase A.
  Attention: 7 k-groups of 448 (one PSUM bank per S tile / pout),
       3 matmuls per (group, h-block): S passA (128ch), S passB (65ch),
       pout with lhsT = [cam | ones | pad] fusing numerator + softmax
       denominator.  pout double-buffered across groups; divide is
       3 DVE ops (stream_shuffle broadcast from PSUM,
       reciprocal_approx_fast, multiply).  fp8 DoubleRow S was tried
       and reverted: the moving operand streams at 1 elem/cycle on HW,
       no faster than 2 fp16 passes.
Inputs deep3/x2/x are fp16; _4 ships as an fp16 H+L pair (same bytes
as fp32, exact to 2^-21).
"""

import os
import sys

sys.path.insert(0, "/opt/trn_rl_repo")

from contextlib import ExitStack

import numpy as np

import concourse.bass as bass
import concourse.tile as tile
from concourse import bacc, mybir
from concourse.bass_utils import run_bass_kernel_spmd


F32 = mybir.dt.float32
BF16 = mybir.dt.bfloat16
F16 = mybir.dt.float16
F8 = mybir.dt.float8e4
AF = mybir.ActivationFunctionType
ALU = mybir.AluOpType

HW = 3136       # 56*56
HWP = 3200      # h padded to 25*128
N_CORES = 8
EPS = 1e-05

_CACHE = {}


def _resize_mat(h_in: int, h_out: int) -> np.ndarray:
    """Dense [h_in, h_out] bilinear align_corners=True resize matrix."""
    ys = np.linspace(0.0, h_in - 1.0, h_out).astype(np.float32)
    y0 = np.floor(ys).astype(np.int64)
    y1 = np.minimum(y0 + 1, h_in - 1)
    w = (ys - y0).astype(np.float32)
    R = np.zeros((h_in, h_out), np.float32)
    for i in range(h_out):
        R[y0[i], i] += 1.0 - w[i]
        R[y1[i], i] += w[i]
    return R


def _resize_coeffs_112() -> tuple[np.ndarray, np.ndarray]:
    """112->56 align-corners taps: out[i] = a[i]*in[2i] + b[i]*in[2i+1];
    a[55]=0, b[55]=1 selects in[111]."""
    ys = np.linspace(0.0, 111.0, 56).astype(np.float64)
    y0 = np.floor(ys).astype(np.int64)
    w = ys - y0
    a = 1.0 - w
    b = w
    a[55], b[55] = 0.0, 1.0
    return a, b


def _combined_resize_coeff() -> np.ndarray:
    """C [128, 56, 112]: C[p, r, w'] = rowc[p, r] * colc[w'] such that
    resize = rowpair-sum(wpair-sum(y2 * C))."""
    a, b = _resize_coeffs_112()
    colc = np.zeros(112, np.float64)
    colc[0::2] = a
    colc[1::2] = b
    rowc = np.zeros((128, 56), np.float64)
    for half in range(2):
        for lr in range(56):
            j = 28 * half + lr // 2
            rowc[64 * half:64 * half + 64, lr] = a[j] if lr % 2 == 0 else b[j]
    return (rowc[:, :, None] * colc[None, None, :]).astype(np.float32)


def _build_program():
    nc = bacc.Bacc(
        "TRN2", target_bir_lowering=False, debug=False, num_devices=N_CORES
    )

    # ---- DRAM I/O (all host-relaid so every DMA slab is per-partition
    # contiguous: 128 big descriptors per trigger) ----
    d_x2 = nc.dram_tensor("x2", [128, 4, 2, 1568], F16, kind="ExternalInput")
    d_x4 = nc.dram_tensor("x4", [128, 2, 4, HW], F16, kind="ExternalInput")
    d_d3 = nc.dram_tensor("d3", [320, HW], F16, kind="ExternalInput")
    d_x = nc.dram_tensor("x", [112, 5376], F16, kind="ExternalInput")
    d_fc8 = nc.dram_tensor("fc8", [128, 4, 2, 65], F16, kind="ExternalInput")
    d_f83T = nc.dram_tensor("f83T", [128, 64], F16, kind="ExternalInput")
    d_f84T = nc.dram_tensor("f84T", [320, 128], F16, kind="ExternalInput")
    d_qkA = nc.dram_tensor("qkA", [128, 385], F16, kind="ExternalInput")
    d_qkB = nc.dram_tensor("qkB", [67, 385], F16, kind="ExternalInput")
    d_rowc = nc.dram_tensor("rowc", [128, 56], F16, kind="ExternalInput")
    d_colc = nc.dram_tensor("colc", [128, 112], F16, kind="ExternalInput")
    d_rh = nc.dram_tensor("rh448", [112, 4, 56], F16, kind="ExternalInput")
    d_ident = nc.dram_tensor("ident", [128, 128], F32, kind="ExternalInput")
    d_out = nc.dram_tensor("out", [4, HW], F32, kind="ExternalOutput")

    # chunking
    ACH = [(i * 392, 392) for i in range(8)]                  # A-phase psum
    NCHP = [(i * 512, 512) for i in range(6)] + [(3072, 128)]  # 3200 qk
    # attention k-groups: 7x448 (one PSUM bank per S tile and pout)
    KG = [(i * 448, 448) for i in range(7)]

    with tile.TileContext(nc) as tc, ExitStack() as top:
        wpool = top.enter_context(tc.tile_pool(name="wpool", bufs=1))
        persist = top.enter_context(tc.tile_pool(name="persist", bufs=1))
        small = top.enter_context(tc.tile_pool(name="small", bufs=2))

        # persistent SBUF
        d3_0 = persist.tile([128, HW], F16, tag="d3_0")
        d3_1 = persist.tile([128, HW], F16, tag="d3_1")
        d3_2 = persist.tile([128, HW], F16, tag="d3_2")
        camTall = persist.tile([128, 25, 4], F32, tag="camTall")
        camT = persist.tile([128, 25, 65], BF16, tag="camT")
        f_a = persist.tile([128, HWP], F16, tag="f_a")
        f_b = persist.tile([67, HWP], F16, tag="f_b")
        qA = persist.tile([128, HWP], F16, tag="qA")
        kA = persist.tile([128, HWP], F16, tag="kA")
        qB = persist.tile([65, HWP], F16, tag="qB")
        kB = persist.tile([65, HWP], F16, tag="kB")
        cam = persist.tile([4, HW], F32, tag="cam")

        # ---- weights + xsb + deep3 on the ACT queue (ordered by first
        # use; gpsimd DMA is a slow software path — keep bulk off it) ----
        rh = wpool.tile([112, 4, 56], F16, tag="rh")
        nc.scalar.dma_start(
            rh[:].rearrange("p a b -> p (a b)"),
            d_rh.ap().rearrange("p a b -> p (a b)"),
        )
        f83T = wpool.tile([128, 64], F16, tag="f83T")
        nc.scalar.dma_start(f83T[:], d_f83T.ap())
        fc8T = wpool.tile([128, 4, 2, 65], F16, tag="fc8T")
        nc.scalar.dma_start(
            fc8T[:].rearrange("p a b c -> p (a b c)"),
            d_fc8.ap().rearrange("p a b c -> p (a b c)"),
        )
        ident = wpool.tile([128, 128], F32, tag="ident")
        nc.scalar.dma_start(ident[:], d_ident.ap())
        xsb = wpool.tile([112, 4, 3, 448], F16, tag="xsb")
        nc.scalar.dma_start(
            xsb[:].rearrange("p a b c -> p (a b c)"), d_x.ap()
        )
        f84T_0 = wpool.tile([128, 128], F16, tag="f84T0")
        nc.scalar.dma_start(f84T_0[:], d_f84T.ap()[0:128, :])
        f84T_1 = wpool.tile([128, 128], F16, tag="f84T1")
        nc.scalar.dma_start(f84T_1[:], d_f84T.ap()[128:256, :])
        f84T_2 = wpool.tile([128, 128], F16, tag="f84T2")
        nc.scalar.dma_start(f84T_2[0:64, :], d_f84T.ap()[256:320, :])
        nc.scalar.dma_start(d3_0[:], d_d3.ap()[0:128, :])
        nc.scalar.dma_start(d3_1[:], d_d3.ap()[128:256, :])
        nc.scalar.dma_start(d3_2[0:64, :], d_d3.ap()[256:320, :])
        rowcb = wpool.tile([128, 56], F16, tag="rowcb")
        nc.scalar.dma_start(rowcb[:], d_rowc.ap())
        colcb = wpool.tile([128, 112], F16, tag="colcb")
        nc.scalar.dma_start(colcb[:], d_colc.ap())
        cres = wpool.tile([128, 56, 112], F16, tag="cres")
        nc.vector.tensor_tensor(
            cres[:],
            rowcb[:].unsqueeze(2).broadcast_to([128, 56, 112]),
            colcb[:].unsqueeze(1).broadcast_to([128, 56, 112]),
            op=ALU.mult,
        )
        qkA = wpool.tile([128, 385], F16, tag="qkA")
        nc.scalar.dma_start(qkA[:], d_qkA.ap())
        qkB = wpool.tile([67, 385], F16, tag="qkB")
        nc.scalar.dma_start(qkB[:], d_qkB.ap())

        # zero pads (cheap engines, off critical path)
        nc.vector.memset(camT[:], 0.0)
        nc.vector.memset(f_a[:, HW:HWP], 0.0)
        nc.vector.memset(f_b[:, HW:HWP], 0.0)
        nc.vector.memset(camTall[64:128, 24, :], 0.0)
        nc.vector.memset(qB[64:65, :], 0.0)
        nc.vector.memset(kB[64:65, :], 0.0)
        nc.vector.memset(d3_2[64:128, :], 0.0)
        nc.vector.memset(f84T_2[64:128, :], 0.0)

        mn = small.tile([4, 1], F32, tag="mn")
        mx = small.tile([4, 1], F32, tag="mx")

        # ============ phase A ============
        with tc.tile_pool(name="inA2", bufs=2) as inA2, \
             tc.tile_pool(name="inA4", bufs=4) as inA4, \
             tc.tile_pool(name="sbA", bufs=1) as sbA, \
             tc.tile_pool(name="pXs", bufs=2, space=bass.MemorySpace.PSUM) as pXs, \
             tc.tile_pool(name="pAy", bufs=2, space=bass.MemorySpace.PSUM) as pAy, \
             tc.tile_pool(name="pAc", bufs=3, space=bass.MemorySpace.PSUM) as pAc:
            # phase-A-lifetime SBUF
            y2 = sbA.tile([128, 56, 112], F16, tag="y2")
            x2ts, x4ts = [], []

            def x2chunk(j):
                t = inA2.tile([128, 2, 1568], F16, tag="x2c")
                nc.sync.dma_start(
                    t[:].rearrange("p a b -> p (a b)"),
                    d_x2.ap()[:, j].rearrange("p a b -> p (a b)"),
                )
                x2ts.append(t)

            def x4chunk(j):
                t = inA4.tile([128, 2, 4, 784], F16, tag="x4c")
                nc.sync.dma_start(
                    t[:], d_x4.ap()[:, :, :, 784 * j:784 * (j + 1)]
                )
                x4ts.append(t)

            # input triggers: x2/x4 interleaved on SP, consumed in this
            # order
            x2chunk(0)
            x2chunk(1)
            x4chunk(0)
            x2chunk(2)
            x4chunk(1)
            x2chunk(3)
            x4chunk(2)
            x4chunk(3)

            # ---- y2 + cam interleaved, 392-col psum chunks ----
            y2v = y2[:].rearrange("p r w -> p (r w)")

            U = sbA.tile([128, 56, 56], F16, tag="ru")
            R = sbA.tile([128, 28, 56], F16, tag="rr")
            f83p = sbA.tile([128, 28, 56], F16, tag="f83p")

            def resize_part(j):
                # y2 rows 14j..14j+14 (one x2 chunk) -> f8_3 rows 7j..7j+7
                r0, r1 = 14 * j, 14 * (j + 1)
                l0, l1 = 7 * j, 7 * (j + 1)
                nc.vector.tensor_tensor(
                    y2[:, r0:r1, :], y2[:, r0:r1, :], cres[:, r0:r1, :],
                    op=ALU.mult,
                )
                nc.vector.tensor_tensor(
                    U[:, r0:r1, :], y2[:, r0:r1, 0:111:2],
                    y2[:, r0:r1, 1:112:2], op=ALU.add,
                )
                nc.vector.tensor_tensor(
                    R[:, l0:l1, :], U[:, r0:r1:2, :], U[:, r0 + 1:r1:2, :],
                    op=ALU.add,
                )
                nc.scalar.activation(
                    f83p[:, l0:l1, :], R[:, l0:l1, :], AF.Relu
                )
                fv = f83p[:].rearrange("p r w -> p (r w)")
                nc.sync.dma_start(
                    f_b[0:64, 56 * l0:56 * l1], fv[0:64, 56 * l0:56 * l1]
                )
                nc.sync.dma_start(
                    f_b[0:64, 1568 + 56 * l0:1568 + 56 * l1],
                    fv[64:128, 56 * l0:56 * l1],
                )

            def f84chunk(no, nl):
                fp = pAy.tile([128, 392], F32, tag="y2p")
                for ci2, (dt_, wt_) in enumerate(
                    ((d3_0, f84T_0), (d3_1, f84T_1), (d3_2, f84T_2))
                ):
                    nc.tensor.matmul(
                        fp[:], wt_[:], dt_[:, no:no + nl],
                        start=(ci2 == 0), stop=(ci2 == 2),
                    )
                nc.scalar.activation(f_a[:, no:no + nl], fp[:], AF.Relu)

            def y2chunk(j):
                # one x2 DMA chunk = 1568 cols = 4 psum chunks
                x2t = x2ts[j]
                for s in range(4):
                    co = 1568 * j + 392 * s
                    pp = pAy.tile([128, 392], F32, tag="y2p")
                    nc.tensor.matmul(
                        pp[0:64, :], f83T[:], x2t[:, 0, 392 * s:392 * s + 392],
                        start=True, stop=True,
                    )
                    nc.tensor.matmul(
                        pp[64:128, :], f83T[:], x2t[:, 1, 392 * s:392 * s + 392],
                        start=True, stop=True,
                    )
                    if s % 2 == 0:
                        nc.vector.tensor_copy(y2v[:, co:co + 392], pp[:])
                    else:
                        nc.scalar.copy(y2v[:, co:co + 392], pp[:])

            def camchunk(j):
                # one x4 DMA chunk = 784 cols = 2 psum chunks
                x4t = x4ts[j]
                for s in range(2):
                    no = 784 * j + 392 * s
                    cp = pAc.tile([65, 392], F32, tag="campsum")
                    first = True
                    for hl, wl in ((0, 0), (0, 1), (1, 0)):
                        # x4 part hl (H or L) against weight part wl (Wh/Wl)
                        for ck in range(4):
                            nc.tensor.matmul(
                                cp[:], fc8T[:, ck, wl, :],
                                x4t[:, hl, ck, 392 * s:392 * s + 392],
                                start=first,
                                stop=(hl == 1 and ck == 3),
                            )
                            first = False
                    nc.scalar.copy(cam[:, no:no + 392], cp[0:4, :])
                    pmn = small.tile([4, 1], F32, tag="pmn")
                    nc.vector.tensor_reduce(
                        pmn[:], cp[0:4, :], axis=mybir.AxisListType.X, op=ALU.min
                    )
                    pmx = small.tile([4, 1], F32, tag="pmx")
                    nc.vector.tensor_reduce(
                        pmx[:], cp[0:4, :], axis=mybir.AxisListType.X, op=ALU.max
                    )
                    if j == 0 and s == 0:
                        nc.vector.tensor_copy(mn[:], pmn[:])
                        nc.vector.tensor_copy(mx[:], pmx[:])
                    else:
                        nc.vector.tensor_tensor(mn[:], mn[:], pmn[:], op=ALU.min)
                        nc.vector.tensor_tensor(mx[:], mx[:], pmx[:], op=ALU.max)

            y2chunk(0)
            resize_part(0)
            # ---- x_s: needs only xsb + rh ----
            xh = sbA.tile([56, 3, 448], F16, tag="xh")
            for c in range(3):
                xp = pXs.tile([56, 448], F32, tag="xsp")
                for hc in range(4):
                    nc.tensor.matmul(
                        xp[:], rh[:, hc, :], xsb[:, hc, c, :],
                        start=(hc == 0), stop=(hc == 3),
                    )
                nc.scalar.copy(xh[:, c, :], xp[:])
            xhT = sbA.tile([112, 12, 56], F16, tag="xhT")
            idb = sbA.tile([128, 128], F16, tag="idb")
            nc.vector.tensor_copy(idb[:], ident[:])
            for c in range(3):
                for wc in range(4):
                    tp2 = pXs.tile([112, 56], F16, tag="xsp")
                    nc.tensor.transpose(
                        tp2[:], xh[:, c, 112 * wc:112 * (wc + 1)], idb[0:56, 0:56]
                    )
                    nc.scalar.copy(xhT[:, c * 4 + wc, :], tp2[:])
            for c in range(3):
                wp = pXs.tile([56, 56], F32, tag="xsp")
                for wc in range(4):
                    nc.tensor.matmul(
                        wp[:], xhT[:, c * 4 + wc, :], rh[:, wc, :],
                        start=(wc == 0), stop=(wc == 3),
                    )
                ws = sbA.tile([56, 56], F16, tag="xws")
                nc.vector.tensor_copy(ws[:], wp[:])
                nc.sync.dma_start(f_b[64 + c:65 + c, 0:HW], ws[:])

            y2chunk(1)
            resize_part(1)
            camchunk(0)
            y2chunk(2)
            resize_part(2)
            camchunk(1)
            for no, nl in ACH:
                f84chunk(no, nl)
            y2chunk(3)
            resize_part(3)
            camchunk(2)
            camchunk(3)


        # ============ phase B: cam transposes + finalize ; f8_4 ============
        with tc.tile_pool(name="pBsb", bufs=1) as pBsb, \
             tc.tile_pool(name="pTp", bufs=2, space=bass.MemorySpace.PSUM) as pTp:
            # ---- cam -> camTall transposes, 5 h-blocks per psum tile ----
            for g in range(5):
                tp = pTp.tile([128, 5, 4], F32, tag="tpsum")
                for t in range(5):
                    bi = 5 * g + t
                    hl = min(128, HW - 128 * bi)
                    nc.tensor.transpose(
                        tp[0:hl, t, :], cam[:, 128 * bi:128 * bi + hl],
                        ident[0:4, 0:4],
                    )
                if g < 4:
                    nc.vector.tensor_copy(camTall[:, 5 * g:5 * g + 5, :], tp[:])
                else:
                    nc.vector.tensor_copy(
                        camTall[0:64, 20:25, :], tp[0:64, :, :]
                    )
                    nc.vector.tensor_copy(
                        camTall[64:128, 20:24, :], tp[64:128, 0:4, :]
                    )
            # scheduling fence: the static scheduler's DMA model hoists
            # the qk matmuls ahead of cam3, delaying camT and stalling
            # phase D's first pout.  A dummy write to qkA's pad column
            # pins every qk matmul after the cam3->transpose chain.
            nc.vector.memset(qkA[0:1, 384:385], 0.0)

            # ---- camT finalize: normalize on transposed layout ----
            rng = small.tile([4, 1], F32, tag="rng")
            nc.vector.tensor_tensor(rng[:], mx[:], mn[:], op=ALU.subtract)
            nc.vector.tensor_scalar_add(rng[:], rng[:], EPS)
            rs = small.tile([4, 1], F32, tag="rs")
            nc.vector.reciprocal(rs[:], rng[:])
            mrow = small.tile([1, 8], F32, tag="mrow")
            nc.sync.dma_start(mrow[0:1, 0:4], mn[:])
            nc.sync.dma_start(mrow[0:1, 4:8], rs[:])
            # broadcast mrow across partitions: ones[1,128]^T @ mrow[1,8]
            ones1 = small.tile([1, 128], F32, tag="ones1")
            nc.vector.memset(ones1[:], 1.0)
            mbc = pTp.tile([128, 8], F32, tag="mbcp")
            nc.tensor.matmul(mbc[:], ones1[:], mrow[:], start=True, stop=True)
            normT = pBsb.tile([128, 25, 4], F32, tag="normT")
            nc.vector.tensor_tensor(
                normT[:], camTall[:],
                mbc[:, 0:4].unsqueeze(1).broadcast_to([128, 25, 4]),
                op=ALU.subtract,
            )
            nc.vector.tensor_tensor(
                normT[:], normT[:],
                mbc[:, 4:8].unsqueeze(1).broadcast_to([128, 25, 4]),
                op=ALU.mult,
            )
            c5v = camT[:]  # [128, 25, 65]
            nc.vector.memset(c5v[:, :, 4], 1.0)
            fm = pBsb.tile([128, 25], F32, tag="fm")
            nc.vector.tensor_reduce(
                fm[:], normT[:, :, 1:4], axis=mybir.AxisListType.X, op=ALU.max
            )
            nc.vector.tensor_scalar(
                c5v[:, :, 0], fm[:], -1.0, 1.0, op0=ALU.mult, op1=ALU.add
            )
            msk = pBsb.tile([128, 25, 3], F32, tag="msk")
            fmb = fm[:].unsqueeze(2).broadcast_to([128, 25, 3])
            nc.vector.tensor_tensor(msk[:], normT[:, :, 1:4], fmb, op=ALU.is_ge)
            nc.vector.tensor_tensor(
                c5v[:, :, 1:4], normT[:, :, 1:4], msk[:], op=ALU.mult
            )
            # h-pad rows of the last block must contribute nothing
            nc.vector.memset(c5v[64:128, 24, 0:5], 0.0)

        # ============ phase C: q, k ============
        with tc.tile_pool(name="pCp", bufs=6, space=bass.MemorySpace.PSUM) as pCp:
            MCH = [(qA, 0, 128), (qB, 128, 64), (kA, 192, 128), (kB, 320, 64)]
            for no, nl in NCHP:
                for mi, (dst, mo, ml) in enumerate(MCH):
                    mlp = 128 if ml == 128 else 65
                    qp = pCp.tile([128, 512], F32, tag="qkpsum")
                    nc.tensor.matmul(
                        qp[0:mlp, 0:nl], qkA[:, mo:mo + mlp], f_a[:, no:no + nl],
                        start=True, stop=False,
                    )
                    nc.tensor.matmul(
                        qp[0:mlp, 0:nl], qkB[:, mo:mo + mlp], f_b[:, no:no + nl],
                        start=False, stop=True,
                    )
                    if mi % 2 == 0:
                        nc.vector.tensor_copy(dst[0:ml, no:no + nl], qp[0:ml, 0:nl])
                    else:
                        nc.scalar.copy(dst[0:ml, no:no + nl], qp[0:ml, 0:nl])

        # ============ phase D: attention, 7 k-groups of 448 ============
        with tc.tile_pool(name="pDe", bufs=5) as pDe, \
             tc.tile_pool(name="pDr", bufs=2) as pDr, \
             tc.tile_pool(name="pDs", bufs=5, space=bass.MemorySpace.PSUM) as pDs, \
             tc.tile_pool(name="pDo", bufs=2, space=bass.MemorySpace.PSUM) as pDo:
            for ko, kl in KG:
                pout = pDo.tile([65, 448], F32, tag="pout")
                for bi in range(25):
                    ho = 128 * bi
                    sp = pDs.tile([128, 448], F32, tag="spsum")
                    nc.tensor.matmul(
                        sp[:, 0:kl], qA[:, ho:ho + 128], kA[:, ko:ko + kl],
                        start=True, stop=False,
                    )
                    nc.tensor.matmul(
                        sp[:, 0:kl], qB[:, ho:ho + 128], kB[:, ko:ko + kl],
                        start=False, stop=True,
                    )
                    et = pDe.tile([128, 448], BF16, tag="exptile")
                    nc.scalar.activation(et[:, 0:kl], sp[:, 0:kl], AF.Exp)
                    nc.tensor.matmul(
                        pout[:, 0:kl], camT[:, bi, :], et[:, 0:kl],
                        start=(bi == 0), stop=(bi == 24),
                    )
                # ---- softmax divide: 3 DVE ops ----
                rb5 = pDr.tile([5, 448], F32, tag="rb5")
                nc.vector.stream_shuffle(rb5[:, 0:kl], pout[0:5, 0:kl], [4] * 32)
                rcp4 = pDr.tile([4, 448], F32, tag="rcp4")
                nc.vector.reciprocal_approx_fast(rcp4[:, 0:kl], rb5[0:4, 0:kl])
                res = pDr.tile([4, 448], F32, tag="res")
                nc.vector.tensor_tensor(
                    res[:, 0:kl], pout[0:4, 0:kl], rcp4[:, 0:kl], op=ALU.mult
                )
                nc.sync.dma_start(d_out.ap()[:, ko:ko + kl], res[:, 0:kl])

    nc.compile()
    return nc


def _get_program():
    if "nc" not in _CACHE:
        _CACHE["nc"] = _build_program()
    return _CACHE["nc"]


def _host_prep(inputs: dict) -> list[dict]:
    x = np.asarray(inputs["x"], np.float32)
    x2 = np.asarray(inputs["x2"], np.float32)
    deep3 = np.asarray(inputs["deep3"], np.float32)
    _4 = np.asarray(inputs["_4"], np.float32)
    fc8_w = np.asarray(inputs["fc8_w"], np.float32)
    f83_w = np.asarray(inputs["f83_w"], np.float32)
    f84_w = np.asarray(inputs["f84_w"], np.float32)
    f91_w = np.asarray(inputs["f91_w"], np.float32)
    f92_w = np.asarray(inputs["f92_w"], np.float32)

    n = x.shape[0]
    # fc8 as fp16 hi/lo pair, transposed, padded to 65 stationary cols,
    # pre-permuted to the SBUF [128, 4ck, 2hl, 65] layout
    fc8T = fc8_w.T  # [512, 4]
    wh = fc8T.astype(np.float16)
    wl = (fc8T - wh.astype(np.float32)).astype(np.float16)
    fc8hl = np.zeros((512, 2, 65), np.float16)
    fc8hl[:, 0, 0:4] = wh
    fc8hl[:, 1, 0:4] = wl
    fc8hl = np.ascontiguousarray(
        fc8hl.reshape(4, 128, 2, 65).transpose(1, 0, 2, 3)
    )  # [128, 4, 2, 65]

    f83T = np.ascontiguousarray(f83_w.T.astype(np.float16))    # [128, 64]
    f84T = np.ascontiguousarray(f84_w.T.astype(np.float16))    # [320, 128]
    # f channel permutation: [f8_4 (128), f8_3 (64), x_s (3)]
    perm = np.concatenate([np.arange(67, 195), np.arange(3, 67), np.arange(3)])
    wqk = np.concatenate([f91_w, f92_w], axis=0)[:, perm]  # [384, 195]
    wqkT = np.ascontiguousarray(wqk.T)  # [195, 384]
    qkA = np.zeros((128, 385), np.float32)
    qkA[:, 0:384] = wqkT[0:128]
    qkB = np.zeros((67, 385), np.float32)
    qkB[:, 0:384] = wqkT[128:195]
    qkA = qkA.astype(np.float16)
    qkB = qkB.astype(np.float16)
    import ml_dtypes
    a, b = _resize_coeffs_112()
    colc = np.zeros(112, np.float64)
    colc[0::2] = a
    colc[1::2] = b
    rowc = np.zeros((128, 56), np.float64)
    for half in range(2):
        for lr in range(56):
            j = 28 * half + lr // 2
            rowc[64 * half:64 * half + 64, lr] = a[j] if lr % 2 == 0 else b[j]
    rowcb = rowc.astype(np.float16)
    colcb = np.ascontiguousarray(
        np.broadcast_to(colc.astype(np.float16), (128, 112))
    )
    rh448 = np.ascontiguousarray(
        _resize_mat(448, 56).astype(np.float16)
        .reshape(4, 112, 56).transpose(1, 0, 2)
    )  # [112, 4, 56]
    ident = np.eye(128, dtype=np.float32)

    shared = {
        "fc8": fc8hl, "f83T": f83T, "f84T": f84T, "qkA": qkA, "qkB": qkB,
        "rowc": rowcb, "colc": colcb, "rh448": rh448, "ident": ident,
    }
    in_maps = []
    for i in range(n):
        m = dict(shared)
        x4i = np.ascontiguousarray(
            _4[i].reshape(4, 128, HW).transpose(1, 0, 2)
        )  # [128, 4, HW] f32
        h16 = x4i.astype(np.float16)
        l16 = (x4i - h16.astype(np.float32)).astype(np.float16)
        m["x4"] = np.ascontiguousarray(
            np.stack([h16, l16], axis=1)
        )  # [128, 2hl, 4ck, HW]: col-chunk DMAs use 1.5KB descriptors
        m["d3"] = deep3[i].reshape(320, HW).astype(np.float16)
        m["x2"] = np.ascontiguousarray(
            x2[i].reshape(128, 2, 4, 1568).transpose(0, 2, 1, 3)
            .astype(np.float16)
        )  # [128, 4chunk, 2half, 1568]
        m["x"] = np.ascontiguousarray(
            x[i].transpose(1, 0, 2).reshape(4, 112, 3, 448)
            .transpose(1, 0, 2, 3).reshape(112, 5376).astype(np.float16)
        )
        in_maps.append(m)
    return in_maps


def _install_ntff_hook() -> bool:
    """Register the NTFF profile hook that the agent image's antenv lacks."""
    try:
        import types

        import antenv

        if "antenv.axon_hooks" not in sys.modules:
            mod = types.ModuleType("antenv.axon_hooks")
            store = {"h": None}
            mod.set_axon_ntff_profile_hook = lambda h: store.update(h=h)
            mod.get_axon_ntff_profile_hook = lambda: store["h"]
            sys.modules["antenv.axon_hooks"] = mod
            antenv.axon_hooks = mod
            from trn_agent_boot.trn_boot import _ntff_profile_via_ctypes

            hook = _ntff_profile_via_ctypes("/opt/axon/libaxon_pjrt.so")
            if hook is None:
                return False
            mod.set_axon_ntff_profile_hook(hook)
        return sys.modules["antenv.axon_hooks"].get_axon_ntff_profile_hook() is not None
    except Exception as e:  # profiling is best-effort
        print(f"ntff hook install failed: {e}", file=sys.stderr)
        return False


def kernel(**inputs) -> np.ndarray:
    nc = _get_program()
    in_maps = _host_prep(inputs)
    trace = bool(int(os.environ.get("KERNEL_PROFILE", "0")))
    if trace:
        trace = _install_ntff_hook()
    res = run_bass_kernel_spmd(nc, in_maps, core_ids=list(range(N_CORES)),
                               trace=trace)
    _CACHE["last_result"] = res
    out = np.stack([r["out"] for r in res.results]).reshape(8, 4, 56, 56)
    return out.astype(np.float32)


# revision 39
# speedup vs baseline: 1.1560x; 1.1560x over previous
"""Trainium2 Bass kernel for nn_Net_74552042324489.

Data-parallel over batch n=8 across 8 NeuronCores (1 sample/core).
~195us measured (225us baseline).  Per-core pipeline:
  DMA: bulk inputs only on the two hardware DGE queues, interleaved in
       PE-consumption order (SP: x2/x4; ACT: weights -> xsb -> deep3 ->
       qk weights).  Host lays slabs out for 1.5-6KB descriptors (12KB
       descriptors serialize on single DMA engines; gpsimd queue is a
       slow software path used only for tiny SBUF moves).  deep3 pad
       rows are memset on-chip; the 1.6MB resize-coefficient tensor is
       built on-chip from its rank-1 factors (one broadcast multiply).
  cam  in fp16 hi/lo 3-term (Wh*H + Wl*H + Wh*L): fp32-level accuracy
       (needed for the argmax-suppression ties; bf16 pairs measurably
       flip them) at fp16 matmul rate.  x stays fp16: fp8 x pushes the
       output to 1.7e-2 via the q,k cross terms.
  x_s  emitted right after the first y2 chunk as PE filler; identity
       ships via DMA.  The 112->56 resize runs per-x2-chunk so f_b is
       ready as soon as the last x2 chunk lands.  f8_4 reuses the y2
       PSUM pool inside phase A.
  Attention: 7 k-groups of 448 (one PSUM bank per S tile / pout),
       3 matmuls per (group, h-block): S passA (128ch), S passB (65ch),
       pout with lhsT = [cam | ones | pad] fusing numerator + softmax
       denominator.  pout double-buffered across groups; divide is
       3 DVE ops (stream_shuffle broadcast from PSUM,
       reciprocal_approx_fast, multiply).  fp8 DoubleRow S was tried
       and reverted: the moving operand streams at 1 elem/cycle on HW,
       no faster than 2 fp16 passes.
Inputs deep3/x2/x are fp16; _4 ships as an fp16 H+L pair (same bytes
as fp32, exact to 2^-21).
"""

import os
import sys

sys.path.insert(0, "/opt/trn_rl_repo")

from contextlib import ExitStack

import numpy as np

import concourse.bass as bass
import concourse.tile as tile
from concourse import bacc, mybir
from concourse.bass_utils import run_bass_kernel_spmd


F32 = mybir.dt.float32
BF16 = mybir.dt.bfloat16
F16 = mybir.dt.float16
F8 = mybir.dt.float8e4
AF = mybir.ActivationFunctionType
ALU = mybir.AluOpType

HW = 3136       # 56*56
HWP = 3200      # h padded to 25*128
N_CORES = 8
EPS = 1e-05

_CACHE = {}


def _resize_mat(h_in: int, h_out: int) -> np.ndarray:
    """Dense [h_in, h_out] bilinear align_corners=True resize matrix."""
    ys = np.linspace(0.0, h_in - 1.0, h_out).astype(np.float32)
    y0 = np.floor(ys).astype(np.int64)
    y1 = np.minimum(y0 + 1, h_in - 1)
    w = (ys - y0).astype(np.float32)
    R = np.zeros((h_in, h_out), np.float32)
    for i in range(h_out):
        R[y0[i], i] += 1.0 - w[i]
        R[y1[i], i] += w[i]
    return R


def _resize_coeffs_112() -> tuple[np.ndarray, np.ndarray]:
    """112->56 align-corners taps: out[i] = a[i]*in[2i] + b[i]*in[2i+1];
    a[55]=0, b[55]=1 selects in[111]."""
    ys = np.linspace(0.0, 111.0, 56).astype(np.float64)
    y0 = np.floor(ys).astype(np.int64)
    w = ys - y0
    a = 1.0 - w
    b = w
    a[55], b[55] = 0.0, 1.0
    return a, b


def _combined_resize_coeff() -> np.ndarray:
    """C [128, 56, 112]: C[p, r, w'] = rowc[p, r] * colc[w'] such that
    resize = rowpair-sum(wpair-sum(y2 * C))."""
    a, b = _resize_coeffs_112()
    colc = np.zeros(112, np.float64)
    colc[0::2] = a
    colc[1::2] = b
    rowc = np.zeros((128, 56), np.float64)
    for half in range(2):
        for lr in range(56):
            j = 28 * half + lr // 2
            rowc[64 * half:64 * half + 64, lr] = a[j] if lr % 2 == 0 else b[j]
    return (rowc[:, :, None] * colc[None, None, :]).astype(np.float32)


def _build_program():
    nc = bacc.Bacc(
        "TRN2", target_bir_lowering=False, debug=False, num_devices=N_CORES
    )

    # ---- DRAM I/O (all host-relaid so every DMA slab is per-partition
    # contiguous: 128 big descriptors per trigger) ----
    d_x2 = nc.dram_tensor("x2", [128, 4, 2, 1568], F16, kind="ExternalInput")
    d_x4 = nc.dram_tensor("x4", [128, 2, 4, HW], F16, kind="ExternalInput")
    d_d3 = nc.dram_tensor("d3", [320, HW], F16, kind="ExternalInput")
    d_x = nc.dram_tensor("x", [112, 5376], F16, kind="ExternalInput")
    d_fc8 = nc.dram_tensor("fc8", [128, 4, 2, 65], F16, kind="ExternalInput")
    d_f83T = nc.dram_tensor("f83T", [128, 64], F16, kind="ExternalInput")
    d_f84T = nc.dram_tensor("f84T", [320, 128], F16, kind="ExternalInput")
    d_qkA = nc.dram_tensor("qkA", [128, 385], F16, kind="ExternalInput")
    d_qkB = nc.dram_tensor("qkB", [67, 385], F16, kind="ExternalInput")
    d_rowc = nc.dram_tensor("rowc", [128, 56], F16, kind="ExternalInput")
    d_colc = nc.dram_tensor("colc", [128, 112], F16, kind="ExternalInput")
    d_rh = nc.dram_tensor("rh448", [112, 4, 56], F16, kind="ExternalInput")
    d_ident = nc.dram_tensor("ident", [128, 128], F32, kind="ExternalInput")
    d_out = nc.dram_tensor("out", [4, HW], F32, kind="ExternalOutput")

    # chunking
    ACH = [(i * 392, 392) for i in range(8)]                  # A-phase psum
    NCHP = [(i * 512, 512) for i in range(6)] + [(3072, 128)]  # 3200 qk
    # attention k-groups: 7x448 (one PSUM bank per S tile and pout)
    KG = [(i * 448, 448) for i in range(7)]

    with tile.TileContext(nc) as tc, ExitStack() as top:
        wpool = top.enter_context(tc.tile_pool(name="wpool", bufs=1))
        persist = top.enter_context(tc.tile_pool(name="persist", bufs=1))
        small = top.enter_context(tc.tile_pool(name="small", bufs=2))

        # persistent SBUF
        d3_0 = persist.tile([128, HW], F16, tag="d3_0")
        d3_1 = persist.tile([128, HW], F16, tag="d3_1")
        d3_2 = persist.tile([128, HW], F16, tag="d3_2")
        camTall = persist.tile([128, 25, 4], F32, tag="camTall")
        camT = persist.tile([128, 25, 65], BF16, tag="camT")
        f_a = persist.tile([128, HWP], F16, tag="f_a")
        f_b = persist.tile([67, HWP], F16, tag="f_b")
        qA = persist.tile([128, HWP], F16, tag="qA")
        kA = persist.tile([128, HWP], F16, tag="kA")
        qB = persist.tile([65, HWP], F16, tag="qB")
        kB = persist.tile([65, HWP], F16, tag="kB")
        cam = persist.tile([4, HW], F32, tag="cam")

        # ---- weights + xsb + deep3 on the ACT queue (ordered by first
        # use; gpsimd DMA is a slow software path — keep bulk off it) ----
        rh = wpool.tile([112, 4, 56], F16, tag="rh")
        nc.scalar.dma_start(
            rh[:].rearrange("p a b -> p (a b)"),
            d_rh.ap().rearrange("p a b -> p (a b)"),
        )
        f83T = wpool.tile([128, 64], F16, tag="f83T")
        nc.scalar.dma_start(f83T[:], d_f83T.ap())
        fc8T = wpool.tile([128, 4, 2, 65], F16, tag="fc8T")
        nc.scalar.dma_start(
            fc8T[:].rearrange("p a b c -> p (a b c)"),
            d_fc8.ap().rearrange("p a b c -> p (a b c)"),
        )
        ident = wpool.tile([128, 128], F32, tag="ident")
        nc.scalar.dma_start(ident[:], d_ident.ap())
        xsb = wpool.tile([112, 4, 3, 448], F16, tag="xsb")
        nc.scalar.dma_start(
            xsb[:].rearrange("p a b c -> p (a b c)"), d_x.ap()
        )
        f84T_0 = wpool.tile([128, 128], F16, tag="f84T0")
        nc.scalar.dma_start(f84T_0[:], d_f84T.ap()[0:128, :])
        f84T_1 = wpool.tile([128, 128], F16, tag="f84T1")
        nc.scalar.dma_start(f84T_1[:], d_f84T.ap()[128:256, :])
        f84T_2 = wpool.tile([128, 128], F16, tag="f84T2")
        nc.scalar.dma_start(f84T_2[0:64, :], d_f84T.ap()[256:320, :])
        nc.scalar.dma_start(d3_0[:], d_d3.ap()[0:128, :])
        nc.scalar.dma_start(d3_1[:], d_d3.ap()[128:256, :])
        nc.scalar.dma_start(d3_2[0:64, :], d_d3.ap()[256:320, :])
        rowcb = wpool.tile([128, 56], F16, tag="rowcb")
        nc.scalar.dma_start(rowcb[:], d_rowc.ap())
        colcb = wpool.tile([128, 112], F16, tag="colcb")
        nc.scalar.dma_start(colcb[:], d_colc.ap())
        cres = wpool.tile([128, 56, 112], F16, tag="cres")
        nc.vector.tensor_tensor(
            cres[:],
            rowcb[:].unsqueeze(2).broadcast_to([128, 56, 112]),
            colcb[:].unsqueeze(1).broadcast_to([128, 56, 112]),
            op=ALU.mult,
        )
        qkA = wpool.tile([128, 385], F16, tag="qkA")
        nc.scalar.dma_start(qkA[:], d_qkA.ap())
        qkB = wpool.tile([67, 385], F16, tag="qkB")
        nc.scalar.dma_start(qkB[:], d_qkB.ap())

        # zero pads (cheap engines, off critical path)
        nc.vector.memset(camT[:], 0.0)
        nc.vector.memset(f_a[:, HW:HWP], 0.0)
        nc.vector.memset(f_b[:, HW:HWP], 0.0)
        nc.vector.memset(camTall[64:128, 24, :], 0.0)
        nc.vector.memset(qB[64:65, :], 0.0)
        nc.vector.memset(kB[64:65, :], 0.0)
        nc.vector.memset(d3_2[64:128, :], 0.0)
        nc.vector.memset(f84T_2[64:128, :], 0.0)

        mn = small.tile([4, 1], F32, tag="mn")
        mx = small.tile([4, 1], F32, tag="mx")

        # ============ phase A ============
        with tc.tile_pool(name="inA2", bufs=2) as inA2, \
             tc.tile_pool(name="inA4", bufs=4) as inA4, \
             tc.tile_pool(name="sbA", bufs=1) as sbA, \
             tc.tile_pool(name="pXs", bufs=2, space=bass.MemorySpace.PSUM) as pXs, \
             tc.tile_pool(name="pAy", bufs=2, space=bass.MemorySpace.PSUM) as pAy, \
             tc.tile_pool(name="pAc", bufs=3, space=bass.MemorySpace.PSUM) as pAc:
            # phase-A-lifetime SBUF
            y2 = sbA.tile([128, 56, 112], F16, tag="y2")
            x2ts, x4ts = [], []

            def x2chunk(j):
                t = inA2.tile([128, 2, 1568], F16, tag="x2c")
                nc.sync.dma_start(
                    t[:].rearrange("p a b -> p (a b)"),
                    d_x2.ap()[:, j].rearrange("p a b -> p (a b)"),
                )
                x2ts.append(t)

            def x4chunk(j):
                t = inA4.tile([128, 2, 4, 784], F16, tag="x4c")
                nc.sync.dma_start(
                    t[:], d_x4.ap()[:, :, :, 784 * j:784 * (j + 1)]
                )
                x4ts.append(t)

            # input triggers: x2/x4 interleaved on SP, consumed in this
            # order
            x2chunk(0)
            x4chunk(0)
            x2chunk(1)
            x4chunk(1)
            x2chunk(2)
            x4chunk(2)
            x2chunk(3)
            x4chunk(3)

            # ---- y2 + cam interleaved, 392-col psum chunks ----
            y2v = y2[:].rearrange("p r w -> p (r w)")

            U = sbA.tile([128, 56, 56], F16, tag="ru")
            R = sbA.tile([128, 28, 56], F16, tag="rr")
            f83p = sbA.tile([128, 28, 56], F16, tag="f83p")

            def resize_part(j):
                # y2 rows 14j..14j+14 (one x2 chunk) -> f8_3 rows 7j..7j+7
                r0, r1 = 14 * j, 14 * (j + 1)
                l0, l1 = 7 * j, 7 * (j + 1)
                nc.vector.tensor_tensor(
                    y2[:, r0:r1, :], y2[:, r0:r1, :], cres[:, r0:r1, :],
                    op=ALU.mult,
                )
                nc.vector.tensor_tensor(
                    U[:, r0:r1, :], y2[:, r0:r1, 0:111:2],
                    y2[:, r0:r1, 1:112:2], op=ALU.add,
                )
                nc.vector.tensor_tensor(
                    R[:, l0:l1, :], U[:, r0:r1:2, :], U[:, r0 + 1:r1:2, :],
                    op=ALU.add,
                )
                nc.scalar.activation(
                    f83p[:, l0:l1, :], R[:, l0:l1, :], AF.Relu
                )
                fv = f83p[:].rearrange("p r w -> p (r w)")
                nc.sync.dma_start(
                    f_b[0:64, 56 * l0:56 * l1], fv[0:64, 56 * l0:56 * l1]
                )
                nc.sync.dma_start(
                    f_b[0:64, 1568 + 56 * l0:1568 + 56 * l1],
                    fv[64:128, 56 * l0:56 * l1],
                )

            def f84chunk(no, nl):
                fp = pAy.tile([128, 392], F32, tag="y2p")
                for ci2, (dt_, wt_) in enumerate(
                    ((d3_0, f84T_0), (d3_1, f84T_1), (d3_2, f84T_2))
                ):
                    nc.tensor.matmul(
                        fp[:], wt_[:], dt_[:, no:no + nl],
                        start=(ci2 == 0), stop=(ci2 == 2),
                    )
                nc.scalar.activation(f_a[:, no:no + nl], fp[:], AF.Relu)

            def y2chunk(j):
                # one x2 DMA chunk = 1568 cols = 4 psum chunks
                x2t = x2ts[j]
                for s in range(4):
                    co = 1568 * j + 392 * s
                    pp = pAy.tile([128, 392], F32, tag="y2p")
                    nc.tensor.matmul(
                        pp[0:64, :], f83T[:], x2t[:, 0, 392 * s:392 * s + 392],
                        start=True, stop=True,
                    )
                    nc.tensor.matmul(
                        pp[64:128, :], f83T[:], x2t[:, 1, 392 * s:392 * s + 392],
                        start=True, stop=True,
                    )
                    if s % 2 == 0:
                        nc.vector.tensor_copy(y2v[:, co:co + 392], pp[:])
                    else:
                        nc.scalar.copy(y2v[:, co:co + 392], pp[:])

            def camchunk(j):
                # one x4 DMA chunk = 784 cols = 2 psum chunks
                x4t = x4ts[j]
                for s in range(2):
                    no = 784 * j + 392 * s
                    cp = pAc.tile([65, 392], F32, tag="campsum")
                    first = True
                    for hl, wl in ((0, 0), (0, 1), (1, 0)):
                        # x4 part hl (H or L) against weight part wl (Wh/Wl)
                        for ck in range(4):
                            nc.tensor.matmul(
                                cp[:], fc8T[:, ck, wl, :],
                                x4t[:, hl, ck, 392 * s:392 * s + 392],
                                start=first,
                                stop=(hl == 1 and ck == 3),
                            )
                            first = False
                    nc.scalar.copy(cam[:, no:no + 392], cp[0:4, :])
                    pmn = small.tile([4, 1], F32, tag="pmn")
                    nc.vector.tensor_reduce(
                        pmn[:], cp[0:4, :], axis=mybir.AxisListType.X, op=ALU.min
                    )
                    pmx = small.tile([4, 1], F32, tag="pmx")
                    nc.vector.tensor_reduce(
                        pmx[:], cp[0:4, :], axis=mybir.AxisListType.X, op=ALU.max
                    )
                    if j == 0 and s == 0:
                        nc.vector.tensor_copy(mn[:], pmn[:])
                        nc.vector.tensor_copy(mx[:], pmx[:])
                    else:
                        nc.vector.tensor_tensor(mn[:], mn[:], pmn[:], op=ALU.min)
                        nc.vector.tensor_tensor(mx[:], mx[:], pmx[:], op=ALU.max)

            y2chunk(0)
            resize_part(0)
            # ---- x_s: needs only xsb + rh ----
            xh = sbA.tile([56, 3, 448], F16, tag="xh")
            for c in range(3):
                xp = pXs.tile([56, 448], F32, tag="xsp")
                for hc in range(4):
                    nc.tensor.matmul(
                        xp[:], rh[:, hc, :], xsb[:, hc, c, :],
                        start=(hc == 0), stop=(hc == 3),
                    )
                nc.scalar.copy(xh[:, c, :], xp[:])
            xhT = sbA.tile([112, 12, 56], F16, tag="xhT")
            idb = sbA.tile([128, 128], F16, tag="idb")
            nc.vector.tensor_copy(idb[:], ident[:])
            for c in range(3):
                for wc in range(4):
                    tp2 = pXs.tile([112, 56], F16, tag="xsp")
                    nc.tensor.transpose(
                        tp2[:], xh[:, c, 112 * wc:112 * (wc + 1)], idb[0:56, 0:56]
                    )
                    nc.scalar.copy(xhT[:, c * 4 + wc, :], tp2[:])
            for c in range(3):
                wp = pXs.tile([56, 56], F32, tag="xsp")
                for wc in range(4):
                    nc.tensor.matmul(
                        wp[:], xhT[:, c * 4 + wc, :], rh[:, wc, :],
                        start=(wc == 0), stop=(wc == 3),
                    )
                ws = sbA.tile([56, 56], F16, tag="xws")
                nc.vector.tensor_copy(ws[:], wp[:])
                nc.sync.dma_start(f_b[64 + c:65 + c, 0:HW], ws[:])

            camchunk(0)
            y2chunk(1)
            resize_part(1)
            camchunk(1)
            for no, nl in ACH:
                f84chunk(no, nl)
            y2chunk(2)
            resize_part(2)
            camchunk(2)
            y2chunk(3)
            resize_part(3)
            camchunk(3)


        # ============ phase B: cam transposes + finalize ; f8_4 ============
        with tc.tile_pool(name="pBsb", bufs=1) as pBsb, \
             tc.tile_pool(name="pTp", bufs=2, space=bass.MemorySpace.PSUM) as pTp:
            # ---- cam -> camTall transposes, 5 h-blocks per psum tile ----
            for g in range(5):
                tp = pTp.tile([128, 5, 4], F32, tag="tpsum")
                for t in range(5):
                    bi = 5 * g + t
                    hl = min(128, HW - 128 * bi)
                    nc.tensor.transpose(
                        tp[0:hl, t, :], cam[:, 128 * bi:128 * bi + hl],
                        ident[0:4, 0:4],
                    )
                if g < 4:
                    nc.vector.tensor_copy(camTall[:, 5 * g:5 * g + 5, :], tp[:])
                else:
                    nc.vector.tensor_copy(
                        camTall[0:64, 20:25, :], tp[0:64, :, :]
                    )
                    nc.vector.tensor_copy(
                        camTall[64:128, 20:24, :], tp[64:128, 0:4, :]
                    )
            # scheduling fence: the static scheduler's DMA model hoists
            # the qk matmuls ahead of cam3, delaying camT and stalling
            # phase D's first pout.  A dummy write to qkA's pad column
            # pins every qk matmul after the cam3->transpose chain.
            nc.vector.memset(qkA[0:1, 384:385], 0.0)

            # ---- camT finalize: normalize on transposed layout ----
            rng = small.tile([4, 1], F32, tag="rng")
            nc.vector.tensor_tensor(rng[:], mx[:], mn[:], op=ALU.subtract)
            nc.vector.tensor_scalar_add(rng[:], rng[:], EPS)
            rs = small.tile([4, 1], F32, tag="rs")
            nc.vector.reciprocal(rs[:], rng[:])
            mrow = small.tile([1, 8], F32, tag="mrow")
            nc.sync.dma_start(mrow[0:1, 0:4], mn[:])
            nc.sync.dma_start(mrow[0:1, 4:8], rs[:])
            # broadcast mrow across partitions: ones[1,128]^T @ mrow[1,8]
            ones1 = small.tile([1, 128], F32, tag="ones1")
            nc.vector.memset(ones1[:], 1.0)
            mbc = pTp.tile([128, 8], F32, tag="mbcp")
            nc.tensor.matmul(mbc[:], ones1[:], mrow[:], start=True, stop=True)
            normT = pBsb.tile([128, 25, 4], F32, tag="normT")
            nc.vector.tensor_tensor(
                normT[:], camTall[:],
                mbc[:, 0:4].unsqueeze(1).broadcast_to([128, 25, 4]),
                op=ALU.subtract,
            )
            nc.vector.tensor_tensor(
                normT[:], normT[:],
                mbc[:, 4:8].unsqueeze(1).broadcast_to([128, 25, 4]),
                op=ALU.mult,
            )
            c5v = camT[:]  # [128, 25, 65]
            nc.vector.memset(c5v[:, :, 4], 1.0)
            fm = pBsb.tile([128, 25], F32, tag="fm")
            nc.vector.tensor_reduce(
                fm[:], normT[:, :, 1:4], axis=mybir.AxisListType.X, op=ALU.max
            )
            nc.vector.tensor_scalar(
                c5v[:, :, 0], fm[:], -1.0, 1.0, op0=ALU.mult, op1=ALU.add
            )
            msk = pBsb.tile([128, 25, 3], F32, tag="msk")
            fmb = fm[:].unsqueeze(2).broadcast_to([128, 25, 3])
            nc.vector.tensor_tensor(msk[:], normT[:, :, 1:4], fmb, op=ALU.is_ge)
            nc.vector.tensor_tensor(
                c5v[:, :, 1:4], normT[:, :, 1:4], msk[:], op=ALU.mult
            )
            # h-pad rows of the last block must contribute nothing
            nc.vector.memset(c5v[64:128, 24, 0:5], 0.0)

        # ============ phase C: q, k ============
        with tc.tile_pool(name="pCp", bufs=6, space=bass.MemorySpace.PSUM) as pCp:
            MCH = [(qA, 0, 128), (qB, 128, 64), (kA, 192, 128), (kB, 320, 64)]
            for no, nl in NCHP:
                for mi, (dst, mo, ml) in enumerate(MCH):
                    mlp = 128 if ml == 128 else 65
                    qp = pCp.tile([128, 512], F32, tag="qkpsum")
                    nc.tensor.matmul(
                        qp[0:mlp, 0:nl], qkA[:, mo:mo + mlp], f_a[:, no:no + nl],
                        start=True, stop=False,
                    )
                    nc.tensor.matmul(
                        qp[0:mlp, 0:nl], qkB[:, mo:mo + mlp], f_b[:, no:no + nl],
                        start=False, stop=True,
                    )
                    if mi % 2 == 0:
                        nc.vector.tensor_copy(dst[0:ml, no:no + nl], qp[0:ml, 0:nl])
                    else:
                        nc.scalar.copy(dst[0:ml, no:no + nl], qp[0:ml, 0:nl])

        # ============ phase D: attention, 7 k-groups of 448 ============
        with tc.tile_pool(name="pDe", bufs=5) as pDe, \
             tc.tile_pool(name="pDr", bufs=2) as pDr, \
             tc.tile_pool(name="pDs", bufs=5, space=bass.MemorySpace.PSUM) as pDs, \
             tc.tile_pool(name="pDo", bufs=2, space=bass.MemorySpace.PSUM) as pDo:
            for ko, kl in KG:
                pout = pDo.tile([65, 448], F32, tag="pout")
                for bi in range(25):
                    ho = 128 * bi
                    sp = pDs.tile([128, 448], F32, tag="spsum")
                    nc.tensor.matmul(
                        sp[:, 0:kl], qA[:, ho:ho + 128], kA[:, ko:ko + kl],
                        start=True, stop=False,
                    )
                    nc.tensor.matmul(
                        sp[:, 0:kl], qB[:, ho:ho + 128], kB[:, ko:ko + kl],
                        start=False, stop=True,
                    )
                    et = pDe.tile([128, 448], BF16, tag="exptile")
                    nc.scalar.activation(et[:, 0:kl], sp[:, 0:kl], AF.Exp)
                    nc.tensor.matmul(
                        pout[:, 0:kl], camT[:, bi, :], et[:, 0:kl],
                        start=(bi == 0), stop=(bi == 24),
                    )
                # ---- softmax divide: 3 DVE ops ----
                rb5 = pDr.tile([5, 448], F32, tag="rb5")
                nc.vector.stream_shuffle(rb5[:, 0:kl], pout[0:5, 0:kl], [4] * 32)
                rcp4 = pDr.tile([4, 448], F32, tag="rcp4")
                nc.vector.reciprocal_approx_fast(rcp4[:, 0:kl], rb5[0:4, 0:kl])
                res = pDr.tile([4, 448], F32, tag="res")
                nc.vector.tensor_tensor(
                    res[:, 0:kl], pout[0:4, 0:kl], rcp4[:, 0:kl], op=ALU.mult
                )
                nc.sync.dma_start(d_out.ap()[:, ko:ko + kl], res[:, 0:kl])

    nc.compile()
    return nc


def _get_program():
    if "nc" not in _CACHE:
        _CACHE["nc"] = _build_program()
    return _CACHE["nc"]


def _host_prep(inputs: dict) -> list[dict]:
    x = np.asarray(inputs["x"], np.float32)
    x2 = np.asarray(inputs["x2"], np.float32)
    deep3 = np.asarray(inputs["deep3"], np.float32)
    _4 = np.asarray(inputs["_4"], np.float32)
    fc8_w = np.asarray(inputs["fc8_w"], np.float32)
    f83_w = np.asarray(inputs["f83_w"], np.float32)
    f84_w = np.asarray(inputs["f84_w"], np.float32)
    f91_w = np.asarray(inputs["f91_w"], np.float32)
    f92_w = np.asarray(inputs["f92_w"], np.float32)

    n = x.shape[0]
    # fc8 as fp16 hi/lo pair, transposed, padded to 65 stationary cols,
    # pre-permuted to the SBUF [128, 4ck, 2hl, 65] layout
    fc8T = fc8_w.T  # [512, 4]
    wh = fc8T.astype(np.float16)
    wl = (fc8T - wh.astype(np.float32)).astype(np.float16)
    fc8hl = np.zeros((512, 2, 65), np.float16)
    fc8hl[:, 0, 0:4] = wh
    fc8hl[:, 1, 0:4] = wl
    fc8hl = np.ascontiguousarray(
        fc8hl.reshape(4, 128, 2, 65).transpose(1, 0, 2, 3)
    )  # [128, 4, 2, 65]

    f83T = np.ascontiguousarray(f83_w.T.astype(np.float16))    # [128, 64]
    f84T = np.ascontiguousarray(f84_w.T.astype(np.float16))    # [320, 128]
    # f channel permutation: [f8_4 (128), f8_3 (64), x_s (3)]
    perm = np.concatenate([np.arange(67, 195), np.arange(3, 67), np.arange(3)])
    wqk = np.concatenate([f91_w, f92_w], axis=0)[:, perm]  # [384, 195]
    wqkT = np.ascontiguousarray(wqk.T)  # [195, 384]
    qkA = np.zeros((128, 385), np.float32)
    qkA[:, 0:384] = wqkT[0:128]
    qkB = np.zeros((67, 385), np.float32)
    qkB[:, 0:384] = wqkT[128:195]
    qkA = qkA.astype(np.float16)
    qkB = qkB.astype(np.float16)
    import ml_dtypes
    a, b = _resize_coeffs_112()
    colc = np.zeros(112, np.float64)
    colc[0::2] = a
    colc[1::2] = b
    rowc = np.zeros((128, 56), np.float64)
    for half in range(2):
        for lr in range(56):
            j = 28 * half + lr // 2
            rowc[64 * half:64 * half + 64, lr] = a[j] if lr % 2 == 0 else b[j]
    rowcb = rowc.astype(np.float16)
    colcb = np.ascontiguousarray(
        np.broadcast_to(colc.astype(np.float16), (128, 112))
    )
    rh448 = np.ascontiguousarray(
        _resize_mat(448, 56).astype(np.float16)
        .reshape(4, 112, 56).transpose(1, 0, 2)
    )  # [112, 4, 56]
    ident = np.eye(128, dtype=np.float32)

    shared = {
        "fc8": fc8hl, "f83T": f83T, "f84T": f84T, "qkA": qkA, "qkB": qkB,
        "rowc": rowcb, "colc": colcb, "rh448": rh448, "ident": ident,
    }
    in_maps = []
    for i in range(n):
        m = dict(shared)
        x4i = np.ascontiguousarray(
            _4[i].reshape(4, 128, HW).transpose(1, 0, 2)
        )  # [128, 4, HW] f32
        h16 = x4i.astype(np.float16)
        l16 = (x4i - h16.astype(np.float32)).astype(np.float16)
        m["x4"] = np.ascontiguousarray(
            np.stack([h16, l16], axis=1)
        )  # [128, 2hl, 4ck, HW]: col-chunk DMAs use 1.5KB descriptors
        m["d3"] = deep3[i].reshape(320, HW).astype(np.float16)
        m["x2"] = np.ascontiguousarray(
            x2[i].reshape(128, 2, 4, 1568).transpose(0, 2, 1, 3)
            .astype(np.float16)
        )  # [128, 4chunk, 2half, 1568]
        m["x"] = np.ascontiguousarray(
            x[i].transpose(1, 0, 2).reshape(4, 112, 3, 448)
            .transpose(1, 0, 2, 3).reshape(112, 5376).astype(np.float16)
        )
        in_maps.append(m)
    return in_maps


def _install_ntff_hook() -> bool:
    """Register the NTFF profile hook that the agent image's antenv lacks."""
    try:
        import types

        import antenv

        if "antenv.axon_hooks" not in sys.modules:
            mod = types.ModuleType("antenv.axon_hooks")
            store = {"h": None}
            mod.set_axon_ntff_profile_hook = lambda h: store.update(h=h)
            mod.get_axon_ntff_profile_hook = lambda: store["h"]
            sys.modules["antenv.axon_hooks"] = mod
            antenv.axon_hooks = mod
            from trn_agent_boot.trn_boot import _ntff_profile_via_ctypes

            hook = _ntff_profile_via_ctypes("/opt/axon/libaxon_pjrt.so")
            if hook is None:
                return False
            mod.set_axon_ntff_profile_hook(hook)
        return sys.modules["antenv.axon_hooks"].get_axon_ntff_profile_hook() is not None
    except Exception as e:  # profiling is best-effort
        print(f"ntff hook install failed: {e}", file=sys.stderr)
        return False


def kernel(**inputs) -> np.ndarray:
    nc = _get_program()
    in_maps = _host_prep(inputs)
    trace = bool(int(os.environ.get("KERNEL_PROFILE", "0")))
    if trace:
        trace = _install_ntff_hook()
    res = run_bass_kernel_spmd(nc, in_maps, core_ids=list(range(N_CORES)),
                               trace=trace)
    _CACHE["last_result"] = res
    out = np.stack([r["out"] for r in res.results]).reshape(8, 4, 56, 56)
    return out.astype(np.float32)
